# revision 18
# baseline (speedup 1.0000x reference)
"""Cross-attention kernel v4 for Trainium2 (Bass/Tile), data-parallel over batch.

Per core: query [1024,1024], context [2048,768] -> out [1024,1024].

Changes vs v2 (346us -> 214us measured):
  - kv tiles processed in PAIRS in the attention loop: the two row-tiled
    score matmul pairs are batched together, then the two exps, then the
    four attn@v matmuls -> half as many PE tiling-mode transitions.
  - softmax normalization uses nc.vector.reciprocal_approx_fast (single
    custom-DVE instruction, ~51 ULP) instead of nc.vector.reciprocal,
    which on real HW is an iterative ~6 cycle/element op that put ~100us+
    of DVE time on the critical path.
  - all matmuls bf16; weights converted once; exp on [128,1024] ACT tiles.
  - PSUM: psS 2x[128,2x512] (4 banks) + psO [65,512]x2 (2) + psK (1) +
    weave (1) = 8 banks.  (Matmul psum outputs must stay within one 2KB
    bank -> all matmul N<=512 fp32.)
  - B (q-proj), C (v-proj), kproj and E (out-proj) chains woven into the
    attention loop through the spare weave bank.

Measured-HW notes (don't regress these):
  - DVE partition-SHIFTED reads work for standard ops but silently produce
    garbage for custom-DVE ops (reciprocal_approx_*).  Keep custom-op APs
    at their natural base partition.
  - Adding work to the scores->exp->attn@v dependency chain on DVE/Pool
    (e.g. Schraudolph exp on DVE, f32->bf16 converts feeding transposes)
    regressed wall time by ~40%: those engines' per-instruction latency is
    far higher than the cost model suggests.  Keep ACT as the only exp
    engine and PE fed straight from DMA'd inputs.
"""

import numpy as np

import concourse.bass as bass
import concourse.tile as tile
from concourse import bacc, mybir
from concourse.alu_op_type import AluOpType
from concourse.bass_utils import run_bass_kernel_spmd
from concourse.masks import make_identity

NQ, QD, CD, NKV = 1024, 1024, 768, 2048
H, DH, INNER = 16, 64, 1024
SCALE = DH**-0.5
NQT, QDT, CDT, KVT, IT = NQ // 128, QD // 128, CD // 128, NKV // 128, INNER // 128
B = 8

f32 = mybir.dt.float32
f32r = mybir.dt.float32r
bf16 = mybir.dt.bfloat16
i16 = mybir.dt.int16
FT = mybir.ActivationFunctionType

# Schraudolph exp on DVE for a subset of kv tiles (offloads the ACT engine).
# E_bits(bf16) = round(128*(s*SCALE*log2(e) + 127 - sigma)); the constant
# sigma / rounding-mode bias is a pure scale factor on exp and cancels in the
# softmax normalization.
SCH_A = 128.0 * SCALE * 1.4426950408889634
SCH_B = 128.0 * (127.0 - 0.0430)
DVE_EXP_EVERY = 0  # kv % N == N-1 tiles go to DVE; 0 disables
BF16_TRANSPOSE = False  # phase-A transposes in bf16 (regressed on HW: the
# f32->bf16 convert copies put Pool/ACT latency on the transpose dep chain)


def declare(nc):
    return dict(
        query=nc.dram_tensor("query", [NQ, QD], f32, kind="ExternalInput"),
        context=nc.dram_tensor("context", [NKV, CD], f32, kind="ExternalInput"),
        w_q=nc.dram_tensor("w_q", [QD, INNER], f32, kind="ExternalInput"),
        w_kv=nc.dram_tensor("w_kv", [CD, 2 * INNER], f32, kind="ExternalInput"),
        w_out=nc.dram_tensor("w_out", [INNER, QD], f32, kind="ExternalInput"),
        b_out=nc.dram_tensor("b_out", [QD], f32, kind="ExternalInput"),
        out=nc.dram_tensor("out", [NQ, QD], f32, kind="ExternalOutput"),
    )


def emit(nc, tc, T):
    query, context, w_q, w_kv = T["query"], T["context"], T["w_q"], T["w_kv"]
    w_out, b_out, out = T["w_out"], T["b_out"], T["out"]

    const = tc.alloc_tile_pool(name="const", bufs=1)
    ident_f = const.tile([128, 128], f32, name="ident_f", tag="ident_f")
    make_identity(nc, ident_f)
    ident = const.tile([128, 128], f32r, name="ident", tag="ident")
    nc.vector.tensor_copy(ident, ident_f)
    if BF16_TRANSPOSE:
        ident_b = const.tile([128, 128], bf16, name="ident_b", tag="ident_b")
        nc.vector.tensor_copy(ident_b, ident_f)
    ones64_f = const.tile([128, 64], f32, name="ones64_f", tag="ones64_f")
    nc.vector.memset(ones64_f, 1.0)
    ones64 = const.tile([128, 64], f32r, name="ones64", tag="ones64")
    nc.vector.tensor_copy(ones64, ones64_f)
    bias_bc = const.tile([128, QD], f32, name="bias", tag="bias")
    nc.sync.dma_start(bias_bc, b_out[:].partition_broadcast(128))

    # ---- stack order: const, OTp, wvop survive into phase E; the rest
    # (qTp..normp) are released LIFO before it. ----
    OTp = tc.alloc_tile_pool(name="OTp", bufs=1)
    OT = [OTp.tile([128, NQ], bf16, name=f"OT{t}", tag=f"OT{t}") for t in range(IT)]
    wvop = tc.alloc_tile_pool(name="wvop", bufs=1)
    wv = [
        wvop.tile([128, INNER], bf16, name=f"wv{j}", tag=f"wv{j}")
        for j in range(CDT)
    ]
    wo = [wvop.tile([128, QD], bf16, name=f"wo{i}", tag=f"wo{i}") for i in range(IT)]

    qTp = tc.alloc_tile_pool(name="qTp", bufs=1)
    qT = [qTp.tile([128, NQ], bf16, name=f"qT{m}", tag=f"qT{m}") for m in range(IT)]
    ctxTp = tc.alloc_tile_pool(name="ctxTp", bufs=1)
    ctxT = [
        ctxTp.tile([128, NKV], bf16, name=f"ctxT{j}", tag=f"ctxT{j}")
        for j in range(CDT)
    ]
    vp = tc.alloc_tile_pool(name="vp", bufs=1)
    v_sb = [
        vp.tile([128, H * 65], bf16, name=f"v{t}", tag=f"v{t}") for t in range(KVT)
    ]
    wfp = tc.alloc_tile_pool(name="wfp", bufs=3)
    wqkp = tc.alloc_tile_pool(name="wqkp", bufs=2)

    def stage(shape, src_ap, dst, eng):
        s = wfp.tile([128, 1024], f32, name="wst", tag="wst")
        sv = s[:, : shape[1] * shape[2]].rearrange(
            "p (a b) -> p a b", a=shape[1]
        ) if len(shape) == 3 else s[:, : shape[1]]
        nc.sync.dma_start(sv, src_ap)
        eng.tensor_copy(dst, sv)

    def load_wv(j, eng):
        stage(
            [128, INNER],
            w_kv[j * 128 : (j + 1) * 128, INNER : 2 * INNER],
            wv[j],
            eng,
        )

    def load_wo(i, eng):
        stage([128, QD], w_out[i * 128 : (i + 1) * 128, :], wo[i], eng)

    def load_wq(m, eng):
        t = wqkp.tile([128, QDT, 128], bf16, name="wqb", tag="wqb")
        src = bass.AP(
            tensor=w_q,
            offset=m * 128,
            ap=[[INNER, 128], [128 * INNER, QDT], [1, 128]],
        )
        stage([128, QDT, 128], src, t, eng)
        return t

    def load_wk(t_, eng):
        t = wqkp.tile([128, CDT, 128], bf16, name="wkb", tag="wkb")
        src = bass.AP(
            tensor=w_kv,
            offset=t_ * 128,
            ap=[[2 * INNER, 128], [128 * 2 * INNER, CDT], [1, 128]],
        )
        stage([128, CDT, 128], src, t, eng)
        return t

    # ---- Phase A: load & transpose query and context ----
    qryTp = tc.alloc_tile_pool(name="qryTp", bufs=1)
    queryT = [
        qryTp.tile([128, NQ], bf16, name=f"qryT{j}", tag=f"qryT{j}")
        for j in range(QDT)
    ]
    cur_wq, cur_wk = {}, {}
    with (
        tc.tile_pool(name="phA", bufs=2) as phA,
        tc.tile_pool(name="psA", bufs=8, space="PSUM") as psA,
    ):
        engs = [nc.vector.tensor_copy, nc.scalar.copy]
        cvt = [nc.scalar.copy, nc.gpsimd.tensor_copy]
        cur_wk[0] = load_wk(0, nc.vector)
        for i in range(KVT):
            if BF16_TRANSPOSE:
                cnat = phA.tile([128, CD], f32, name="cnat", tag="cnat")
                nc.sync.dma_start(cnat, context[i * 128 : (i + 1) * 128, :])
                cnb = phA.tile([128, CD], bf16, name="cnb", tag="cnb")
                cvt[i % 2](cnb, cnat)
                for j in range(CDT):
                    pt = psA.tile([128, 128], bf16, name="ptb", tag="ptb")
                    nc.tensor.transpose(pt, cnb[:, j * 128 : (j + 1) * 128], ident_b)
                    engs[(i + j) % 2](ctxT[j][:, i * 128 : (i + 1) * 128], pt)
            else:
                cnat = phA.tile([128, CD], f32r, name="cnat", tag="cnat")
                nc.sync.dma_start(
                    cnat, context[i * 128 : (i + 1) * 128, :].bitcast(f32r)
                )
                for j in range(CDT):
                    pt = psA.tile([128, 128], f32r, name="pt", tag="pt")
                    nc.tensor.transpose(pt, cnat[:, j * 128 : (j + 1) * 128], ident)
                    engs[(i + j) % 2](ctxT[j][:, i * 128 : (i + 1) * 128], pt)
        for j in range(CDT):
            load_wv(j, (nc.vector, nc.gpsimd)[j % 2])
        for i in range(NQT):
            if BF16_TRANSPOSE:
                qnat = phA.tile([128, QD], f32, name="qnat", tag="qnat")
                nc.sync.dma_start(qnat, query[i * 128 : (i + 1) * 128, :])
                qnb = phA.tile([128, QD], bf16, name="qnb", tag="qnb")
                cvt[i % 2](qnb, qnat)
                for j in range(QDT):
                    pt = psA.tile([128, 128], bf16, name="ptb", tag="ptb")
                    nc.tensor.transpose(pt, qnb[:, j * 128 : (j + 1) * 128], ident_b)
                    engs[(i + j) % 2](queryT[j][:, i * 128 : (i + 1) * 128], pt)
            else:
                qnat = phA.tile([128, QD], f32r, name="qnat", tag="qnat")
                nc.sync.dma_start(
                    qnat, query[i * 128 : (i + 1) * 128, :].bitcast(f32r)
                )
                for j in range(QDT):
                    pt = psA.tile([128, 128], f32r, name="pt", tag="pt")
                    nc.tensor.transpose(pt, qnat[:, j * 128 : (j + 1) * 128], ident)
                    engs[(i + j) % 2](queryT[j][:, i * 128 : (i + 1) * 128], pt)
        for i in range(IT):
            load_wo(i, (nc.vector, nc.gpsimd)[i % 2])
        cur_wq[0] = load_wq(0, nc.vector)
        cur_wq[1] = load_wq(1, nc.gpsimd)

    # ---- weave machinery ----
    wvx = tc.alloc_tile_pool(name="wvx", bufs=1, space="PSUM")
    psK = tc.alloc_tile_pool(name="psK", bufs=1, space="PSUM")
    kTp = tc.alloc_tile_pool(name="kTp", bufs=2)

    def emit_B_half(m, h):
        ps = wvx.tile([128, 512], f32, name="ps_w", tag="ps_w")
        for j in range(QDT):
            nc.tensor.matmul(
                ps,
                cur_wq[m][:, j, :],
                queryT[j][:, h * 512 : (h + 1) * 512],
                start=(j == 0),
                stop=(j == QDT - 1),
            )
        nc.vector.tensor_copy(qT[m][:, h * 512 : (h + 1) * 512], ps)

    def emit_C_chain(kv, h):
        ps = wvx.tile([128, 512], f32, name="ps_w", tag="ps_w")
        for j in range(CDT):
            nc.tensor.matmul(
                ps,
                ctxT[j][:, kv * 128 : (kv + 1) * 128],
                wv[j][:, h * 512 : (h + 1) * 512],
                start=(j == 0),
                stop=(j == CDT - 1),
            )
        vt = v_sb[kv].rearrange("p (h c) -> p h c", c=65)
        nc.vector.tensor_copy(
            vt[:, h * 8 : (h + 1) * 8, 0:64],
            ps.rearrange("p (h c) -> p h c", c=64),
        )
        nc.vector.memset(vt[:, h * 8 : (h + 1) * 8, 64:65], 1.0)

    def emit_kproj_chain(t, kT_t, n):
        ps = psK.tile([128, 512], f32, name="ps_k", tag="ps_k")
        for j in range(CDT):
            nc.tensor.matmul(
                ps,
                cur_wk[t][:, j, :],
                ctxT[j][:, n * 512 : (n + 1) * 512],
                start=(j == 0),
                stop=(j == CDT - 1),
            )
        nc.vector.tensor_copy(kT_t[:, n * 512 : (n + 1) * 512], ps)

    kT_tiles = {}

    def make_kT(t):
        kT_tiles[t] = kTp.tile([128, NKV], bf16, name=f"kT{t}", tag="kT")
        return kT_tiles[t]

    # upfront: qT[0], qT[1], kT[0]
    kT0 = make_kT(0)
    for h in range(2):
        emit_B_half(0, h)
    for n in range(4):
        emit_kproj_chain(0, kT0, n)
    for h in range(2):
        emit_B_half(1, h)

    def weave_queue(t):
        q = []
        if t < IT - 1:
            kt = make_kT(t + 1)
            for n in range(4):
                q.append(lambda n=n, t=t, kt=kt: emit_kproj_chain(t + 1, kt, n))
        if t + 2 < IT:
            for h in range(2):
                q.append(lambda h=h, t=t: emit_B_half(t + 2, h))
        if t < 4:
            for kv in range(4 * t, 4 * t + 4):
                q.append(lambda kv=kv: emit_C_chain(kv, 1))
        return q

    # ---- Phase D ----
    Ep = tc.alloc_tile_pool(name="Ep", bufs=4)
    normp = tc.alloc_tile_pool(name="norm", bufs=2)
    psS = tc.alloc_tile_pool(name="psS", bufs=2, space="PSUM")
    psO = tc.alloc_tile_pool(name="psO", bufs=2, space="PSUM")

    def emit_scores(t, half, kv, kT_t):
        lo = half * 512
        kvlo = kv * 128
        pS = psS.tile([128, 2, 512], f32, name="ps_s", tag="ps_s")
        for hi in range(2):
            plo, phi = (0, 64) if hi == 0 else (64, 128)
            tp = (0, 0) if hi == 0 else (64, 0)
            nc.tensor.matmul(
                pS[:, hi, :],
                kT_t[plo:phi, kvlo : kvlo + 128],
                qT[t][plo:phi, lo : lo + 512],
                start=True,
                stop=True,
                tile_position=tp,
            )
        return pS

    for t in range(IT):
        h0 = 2 * t
        kT_t = kT_tiles[t]
        if t + 1 < IT:
            cur_wk[t + 1] = load_wk(t + 1, nc.gpsimd)
        if t + 2 < IT:
            cur_wq[t + 2] = load_wq(t + 2, nc.gpsimd)
        wq_queue = weave_queue(t)
        for half in range(2):
            lo = half * 512
            pO = {}
            for hi in range(2):
                pO[hi] = psO.tile([65, 512], f32, name="ps_o", tag="ps_o")
            # kv pairs: batch the two row-tiled score pairs together, then the
            # exps, then the pO matmuls -> half as many PE tiling-mode switches
            for kvp in range(KVT // 2):
                kvs = (2 * kvp, 2 * kvp + 1)
                if t == 0 and half == 0:
                    for kv in kvs:
                        emit_C_chain(kv, 0)
                pSs = [emit_scores(t, half, kv, kT_t) for kv in kvs]
                Es = []
                for kv, pS in zip(kvs, pSs):
                    if DVE_EXP_EVERY and kv % DVE_EXP_EVERY == DVE_EXP_EVERY - 1:
                        Ei = Ep.tile([128, 2, 512], i16, name="E", tag="E")
                        nc.vector.tensor_scalar(
                            Ei.rearrange("p h c -> p (h c)"),
                            pS.rearrange("p h c -> p (h c)"),
                            SCH_A,
                            SCH_B,
                            op0=AluOpType.mult,
                            op1=AluOpType.add,
                        )
                        E = Ei.bitcast(bf16)
                    else:
                        E = Ep.tile([128, 2, 512], bf16, name="E", tag="E")
                        nc.scalar.activation(
                            E.rearrange("p h c -> p (h c)"),
                            pS.rearrange("p h c -> p (h c)"),
                            FT.Exp,
                            scale=SCALE,
                        )
                    Es.append(E)
                for kv, E in zip(kvs, Es):
                    for hi in range(2):
                        nc.tensor.matmul(
                            pO[hi],
                            v_sb[kv][:, (h0 + hi) * 65 : (h0 + hi) * 65 + 65],
                            E[:, hi, :],
                            start=(kv == 0),
                            stop=(kv == KVT - 1),
                        )
                if wq_queue:
                    wq_queue.pop(0)()

            # normalization: copy psum out, PE-broadcast the ones-column row,
            # approx-reciprocal (1 instr vs ~6 cyc/elem iterative), scale.
            oc = normp.tile([65, 2, 512], f32r, name="oc", tag="oc")
            for hi in range(2):
                nc.vector.tensor_copy(oc[:, hi, :], pO[hi])
            prb = {}
            for hi in range(2):
                prb[hi] = psO.tile([64, 512], f32, name="ps_o", tag="ps_o")
                nc.tensor.matmul(
                    prb[hi],
                    ones64[64:65, :],
                    oc[64:65, hi, :],
                    start=True,
                    stop=True,
                )
            rb = normp.tile([64, 2, 512], f32, name="rb", tag="rb")
            for hi in range(2):
                nc.vector.reciprocal_approx_fast(rb[:, hi, :], prb[hi])
            for hi in range(2):
                dst = OT[t][hi * 64 : hi * 64 + 64, lo : lo + 512]
                nc.vector.tensor_tensor(
                    dst, oc[0:64, hi, :], rb[:, hi, :], op=AluOpType.mult
                )
        while wq_queue:
            wq_queue.pop(0)()

    # SBUF LIFO: normp, Ep, kTp, qryTp, wqkp, wfp, vp, ctxTp, qTp
    normp.release()
    Ep.release()
    kTp.release()
    qryTp.release()
    wqkp.release()
    wfp.release()
    vp.release()
    ctxTp.release()
    qTp.release()
    # PSUM LIFO: psO, psS, psK, wvx
    psO.release()
    psS.release()
    psK.release()
    wvx.release()

    # ---- Phase E: out = O @ w_out + b_out ----
    with (
        tc.tile_pool(name="osb", bufs=3) as osbp,
        tc.tile_pool(name="psE", bufs=4, space="PSUM") as psE,
    ):
        for m in range(NQT):
            o_sb = osbp.tile([128, QD], f32, name="osb", tag="osb")
            for half in range(2):
                lo = half * 512
                ps = psE.tile([128, 512], f32, name="ps_e", tag="ps_e")
                for i in range(IT):
                    nc.tensor.matmul(
                        ps,
                        OT[i][:, m * 128 : (m + 1) * 128],
                        wo[i][:, lo : lo + 512],
                        start=(i == 0),
                        stop=(i == IT - 1),
                    )
                nc.vector.tensor_tensor(
                    o_sb[:, lo : lo + 512],
                    ps,
                    bias_bc[:, lo : lo + 512],
                    op=AluOpType.add,
                )
            nc.gpsimd.dma_start(out[m * 128 : (m + 1) * 128, :], o_sb)

    wvop.release()
    OTp.release()
    const.release()


def build(reps=1):
    nc = bacc.Bacc("TRN2", target_bir_lowering=False, debug=False)
    T = declare(nc)
    with tile.TileContext(nc) as tc:
        for _ in range(reps):
            emit(nc, tc, T)
    nc.compile()
    return nc


_nc_cache = None


def _get_nc():
    global _nc_cache
    if _nc_cache is None:
        _nc_cache = build()
    return _nc_cache


def kernel(query, context, w_q, w_kv, w_out, b_out, **run_kwargs):
    nc = _get_nc()
    query = np.ascontiguousarray(query, dtype=np.float32)
    context = np.ascontiguousarray(context, dtype=np.float32)
    shared = {
        "w_q": np.ascontiguousarray(w_q, dtype=np.float32),
        "w_kv": np.ascontiguousarray(w_kv, dtype=np.float32),
        "w_out": np.ascontiguousarray(w_out, dtype=np.float32),
        "b_out": np.ascontiguousarray(b_out, dtype=np.float32),
    }
    in_maps = [
        {"query": query[b], "context": context[b], **shared} for b in range(B)
    ]
    res = run_bass_kernel_spmd(nc, in_maps, core_ids=list(range(B)), **run_kwargs)
    out = np.stack([res.results[b]["out"] for b in range(B)])
    if run_kwargs:
        kernel.last_result = res
    return out



# revision 19
# speedup vs baseline: 1.3081x; 1.3081x over previous
"""Cross-attention kernel v4 for Trainium2 (Bass/Tile), data-parallel over batch.

Per core: query [1024,1024], context [2048,768] -> out [1024,1024].

Changes vs v2 (346us -> 214us measured):
  - kv tiles processed in PAIRS in the attention loop: the two row-tiled
    score matmul pairs are batched together, then the two exps, then the
    four attn@v matmuls -> half as many PE tiling-mode transitions.
  - softmax normalization uses nc.vector.reciprocal_approx_fast (single
    custom-DVE instruction, ~51 ULP) instead of nc.vector.reciprocal,
    which on real HW is an iterative ~6 cycle/element op that put ~100us+
    of DVE time on the critical path.
  - all matmuls bf16; weights converted once; exp on [128,1024] ACT tiles.
  - PSUM: psS 2x[128,2x512] (4 banks) + psO [65,512]x2 (2) + psK (1) +
    weave (1) = 8 banks.  (Matmul psum outputs must stay within one 2KB
    bank -> all matmul N<=512 fp32.)
  - B (q-proj), C (v-proj), kproj and E (out-proj) chains woven into the
    attention loop through the spare weave bank.

Measured-HW notes (don't regress these):
  - DVE partition-SHIFTED reads work for standard ops but silently produce
    garbage for custom-DVE ops (reciprocal_approx_*).  Keep custom-op APs
    at their natural base partition.
  - Adding work to the scores->exp->attn@v dependency chain on DVE/Pool
    (e.g. Schraudolph exp on DVE, f32->bf16 converts feeding transposes)
    regressed wall time by ~40%: those engines' per-instruction latency is
    far higher than the cost model suggests.  Keep ACT as the only exp
    engine and PE fed straight from DMA'd inputs.
"""

import numpy as np

import concourse.bass as bass
import concourse.tile as tile
from concourse import bacc, mybir
from concourse.alu_op_type import AluOpType
from concourse.bass_utils import run_bass_kernel_spmd
from concourse.masks import make_identity

NQ, QD, CD, NKV = 1024, 1024, 768, 2048
H, DH, INNER = 16, 64, 1024
SCALE = DH**-0.5
NQT, QDT, CDT, KVT, IT = NQ // 128, QD // 128, CD // 128, NKV // 128, INNER // 128
B = 8

f32 = mybir.dt.float32
f32r = mybir.dt.float32r
bf16 = mybir.dt.bfloat16
i16 = mybir.dt.int16
FT = mybir.ActivationFunctionType

# Schraudolph exp on DVE for a subset of kv tiles (offloads the ACT engine).
# E_bits(bf16) = round(128*(s*SCALE*log2(e) + 127 - sigma)); the constant
# sigma / rounding-mode bias is a pure scale factor on exp and cancels in the
# softmax normalization.
SCH_A = 128.0 * SCALE * 1.4426950408889634
SCH_B = 128.0 * (127.0 - 0.0430)
DVE_EXP_EVERY = 0  # kv % N == N-1 tiles go to DVE; 0 disables
BF16_TRANSPOSE = False  # phase-A transposes in bf16 (regressed on HW: the
# f32->bf16 convert copies put Pool/ACT latency on the transpose dep chain)


def declare(nc):
    return dict(
        query=nc.dram_tensor("query", [NQ, QD], f32, kind="ExternalInput"),
        context=nc.dram_tensor("context", [NKV, CD], f32, kind="ExternalInput"),
        w_q=nc.dram_tensor("w_q", [QD, INNER], f32, kind="ExternalInput"),
        w_kv=nc.dram_tensor("w_kv", [CD, 2 * INNER], f32, kind="ExternalInput"),
        w_out=nc.dram_tensor("w_out", [INNER, QD], f32, kind="ExternalInput"),
        b_out=nc.dram_tensor("b_out", [QD], f32, kind="ExternalInput"),
        out=nc.dram_tensor("out", [NQ, QD], f32, kind="ExternalOutput"),
    )


def emit(nc, tc, T):
    query, context, w_q, w_kv = T["query"], T["context"], T["w_q"], T["w_kv"]
    w_out, b_out, out = T["w_out"], T["b_out"], T["out"]

    const = tc.alloc_tile_pool(name="const", bufs=1)
    ident_f = const.tile([128, 128], f32, name="ident_f", tag="ident_f")
    make_identity(nc, ident_f)
    ident = const.tile([128, 128], f32r, name="ident", tag="ident")
    nc.vector.tensor_copy(ident, ident_f)
    if BF16_TRANSPOSE:
        ident_b = const.tile([128, 128], bf16, name="ident_b", tag="ident_b")
        nc.vector.tensor_copy(ident_b, ident_f)
    ones64_f = const.tile([128, 64], f32, name="ones64_f", tag="ones64_f")
    nc.vector.memset(ones64_f, 1.0)
    ones64 = const.tile([128, 64], f32r, name="ones64", tag="ones64")
    nc.vector.tensor_copy(ones64, ones64_f)
    bias_bc = const.tile([128, QD], f32, name="bias", tag="bias")
    nc.sync.dma_start(bias_bc, b_out[:].partition_broadcast(128))

    # ---- stack order: const, OTp, wvop survive into phase E; the rest
    # (qTp..normp) are released LIFO before it. ----
    OTp = tc.alloc_tile_pool(name="OTp", bufs=1)
    OT = [OTp.tile([128, NQ], bf16, name=f"OT{t}", tag=f"OT{t}") for t in range(IT)]
    wvop = tc.alloc_tile_pool(name="wvop", bufs=1)
    wv = [
        wvop.tile([128, INNER], bf16, name=f"wv{j}", tag=f"wv{j}")
        for j in range(CDT)
    ]
    wo = [wvop.tile([128, QD], bf16, name=f"wo{i}", tag=f"wo{i}") for i in range(IT)]

    qTp = tc.alloc_tile_pool(name="qTp", bufs=1)
    qT = [qTp.tile([128, NQ], bf16, name=f"qT{m}", tag=f"qT{m}") for m in range(IT)]
    ctxTp = tc.alloc_tile_pool(name="ctxTp", bufs=1)
    ctxT = [
        ctxTp.tile([128, NKV], bf16, name=f"ctxT{j}", tag=f"ctxT{j}")
        for j in range(CDT)
    ]
    vp = tc.alloc_tile_pool(name="vp", bufs=1)
    v_sb = [
        vp.tile([128, H * 65], bf16, name=f"v{t}", tag=f"v{t}") for t in range(KVT)
    ]
    wfp = tc.alloc_tile_pool(name="wfp", bufs=3)
    wqkp = tc.alloc_tile_pool(name="wqkp", bufs=2)

    def stage(shape, src_ap, dst, eng):
        s = wfp.tile([128, 1024], f32, name="wst", tag="wst")
        sv = s[:, : shape[1] * shape[2]].rearrange(
            "p (a b) -> p a b", a=shape[1]
        ) if len(shape) == 3 else s[:, : shape[1]]
        nc.sync.dma_start(sv, src_ap)
        eng.tensor_copy(dst, sv)

    def load_wv(j, eng):
        stage(
            [128, INNER],
            w_kv[j * 128 : (j + 1) * 128, INNER : 2 * INNER],
            wv[j],
            eng,
        )

    def load_wo(i, eng):
        stage([128, QD], w_out[i * 128 : (i + 1) * 128, :], wo[i], eng)

    def load_wq(m, eng):
        t = wqkp.tile([128, QDT, 128], bf16, name="wqb", tag="wqb")
        src = bass.AP(
            tensor=w_q,
            offset=m * 128,
            ap=[[INNER, 128], [128 * INNER, QDT], [1, 128]],
        )
        stage([128, QDT, 128], src, t, eng)
        return t

    def load_wk(t_, eng):
        t = wqkp.tile([128, CDT, 128], bf16, name="wkb", tag="wkb")
        src = bass.AP(
            tensor=w_kv,
            offset=t_ * 128,
            ap=[[2 * INNER, 128], [128 * 2 * INNER, CDT], [1, 128]],
        )
        stage([128, CDT, 128], src, t, eng)
        return t

    # ---- Phase A: load & transpose query and context ----
    qryTp = tc.alloc_tile_pool(name="qryTp", bufs=1)
    queryT = [
        qryTp.tile([128, NQ], bf16, name=f"qryT{j}", tag=f"qryT{j}")
        for j in range(QDT)
    ]
    cur_wq, cur_wk = {}, {}
    with (
        tc.tile_pool(name="phA", bufs=2) as phA,
        tc.tile_pool(name="psA", bufs=8, space="PSUM") as psA,
    ):
        engs = [nc.vector.tensor_copy, nc.scalar.copy]
        cvt = [nc.scalar.copy, nc.gpsimd.tensor_copy]
        cur_wk[0] = load_wk(0, nc.vector)
        for i in range(KVT):
            if BF16_TRANSPOSE:
                cnat = phA.tile([128, CD], f32, name="cnat", tag="cnat")
                nc.sync.dma_start(cnat, context[i * 128 : (i + 1) * 128, :])
                cnb = phA.tile([128, CD], bf16, name="cnb", tag="cnb")
                cvt[i % 2](cnb, cnat)
                for j in range(CDT):
                    pt = psA.tile([128, 128], bf16, name="ptb", tag="ptb")
                    nc.tensor.transpose(pt, cnb[:, j * 128 : (j + 1) * 128], ident_b)
                    engs[(i + j) % 2](ctxT[j][:, i * 128 : (i + 1) * 128], pt)
            else:
                cnat = phA.tile([128, CD], f32r, name="cnat", tag="cnat")
                nc.sync.dma_start(
                    cnat, context[i * 128 : (i + 1) * 128, :].bitcast(f32r)
                )
                for j in range(CDT):
                    pt = psA.tile([128, 128], f32r, name="pt", tag="pt")
                    nc.tensor.transpose(pt, cnat[:, j * 128 : (j + 1) * 128], ident)
                    engs[(i + j) % 2](ctxT[j][:, i * 128 : (i + 1) * 128], pt)
        for j in range(CDT):
            load_wv(j, (nc.vector, nc.gpsimd)[j % 2])
        for i in range(NQT):
            if BF16_TRANSPOSE:
                qnat = phA.tile([128, QD], f32, name="qnat", tag="qnat")
                nc.sync.dma_start(qnat, query[i * 128 : (i + 1) * 128, :])
                qnb = phA.tile([128, QD], bf16, name="qnb", tag="qnb")
                cvt[i % 2](qnb, qnat)
                for j in range(QDT):
                    pt = psA.tile([128, 128], bf16, name="ptb", tag="ptb")
                    nc.tensor.transpose(pt, qnb[:, j * 128 : (j + 1) * 128], ident_b)
                    engs[(i + j) % 2](queryT[j][:, i * 128 : (i + 1) * 128], pt)
            else:
                qnat = phA.tile([128, QD], f32r, name="qnat", tag="qnat")
                nc.sync.dma_start(
                    qnat, query[i * 128 : (i + 1) * 128, :].bitcast(f32r)
                )
                for j in range(QDT):
                    pt = psA.tile([128, 128], f32r, name="pt", tag="pt")
                    nc.tensor.transpose(pt, qnat[:, j * 128 : (j + 1) * 128], ident)
                    engs[(i + j) % 2](queryT[j][:, i * 128 : (i + 1) * 128], pt)
        for i in range(IT):
            load_wo(i, (nc.vector, nc.gpsimd)[i % 2])
        cur_wq[0] = load_wq(0, nc.vector)
        cur_wq[1] = load_wq(1, nc.gpsimd)

    # ---- weave machinery ----
    wvx = tc.alloc_tile_pool(name="wvx", bufs=1, space="PSUM")
    psK = tc.alloc_tile_pool(name="psK", bufs=1, space="PSUM")
    kTp = tc.alloc_tile_pool(name="kTp", bufs=2)

    def emit_B_half(m, h):
        ps = wvx.tile([128, 512], f32, name="ps_w", tag="ps_w")
        for j in range(QDT):
            nc.tensor.matmul(
                ps,
                cur_wq[m][:, j, :],
                queryT[j][:, h * 512 : (h + 1) * 512],
                start=(j == 0),
                stop=(j == QDT - 1),
            )
        nc.vector.tensor_copy(qT[m][:, h * 512 : (h + 1) * 512], ps)

    def emit_C_chain(kv, h):
        ps = wvx.tile([128, 512], f32, name="ps_w", tag="ps_w")
        for j in range(CDT):
            nc.tensor.matmul(
                ps,
                ctxT[j][:, kv * 128 : (kv + 1) * 128],
                wv[j][:, h * 512 : (h + 1) * 512],
                start=(j == 0),
                stop=(j == CDT - 1),
            )
        vt = v_sb[kv].rearrange("p (h c) -> p h c", c=65)
        nc.vector.tensor_copy(
            vt[:, h * 8 : (h + 1) * 8, 0:64],
            ps.rearrange("p (h c) -> p h c", c=64),
        )
        nc.vector.memset(vt[:, h * 8 : (h + 1) * 8, 64:65], 1.0)

    def emit_kproj_chain(t, kT_t, n):
        ps = psK.tile([128, 512], f32, name="ps_k", tag="ps_k")
        for j in range(CDT):
            nc.tensor.matmul(
                ps,
                cur_wk[t][:, j, :],
                ctxT[j][:, n * 512 : (n + 1) * 512],
                start=(j == 0),
                stop=(j == CDT - 1),
            )
        nc.vector.tensor_copy(kT_t[:, n * 512 : (n + 1) * 512], ps)

    kT_tiles = {}

    def make_kT(t):
        kT_tiles[t] = kTp.tile([128, NKV], bf16, name=f"kT{t}", tag="kT")
        return kT_tiles[t]

    # upfront: qT[0], qT[1], kT[0]
    kT0 = make_kT(0)
    for h in range(2):
        emit_B_half(0, h)
    for n in range(4):
        emit_kproj_chain(0, kT0, n)
    for h in range(2):
        emit_B_half(1, h)

    def weave_queue(t):
        q = []
        if t < IT - 1:
            kt = make_kT(t + 1)
            for n in range(4):
                q.append(lambda n=n, t=t, kt=kt: emit_kproj_chain(t + 1, kt, n))
        if t + 2 < IT:
            for h in range(2):
                q.append(lambda h=h, t=t: emit_B_half(t + 2, h))
        if t < 4:
            for kv in range(4 * t, 4 * t + 4):
                q.append(lambda kv=kv: emit_C_chain(kv, 1))
        return q

    # ---- Phase D ----
    Ep = tc.alloc_tile_pool(name="Ep", bufs=4)
    normp = tc.alloc_tile_pool(name="norm", bufs=2)
    psS = tc.alloc_tile_pool(name="psS", bufs=2, space="PSUM")
    psO = tc.alloc_tile_pool(name="psO", bufs=2, space="PSUM")

    def emit_scores(t, half, kv, kT_t):
        lo = half * 512
        kvlo = kv * 128
        pS = psS.tile([128, 2, 512], f32, name="ps_s", tag="ps_s")
        for hi in range(2):
            plo, phi = (0, 64) if hi == 0 else (64, 128)
            tp = (0, 0) if hi == 0 else (64, 0)
            nc.tensor.matmul(
                pS[:, hi, :],
                kT_t[plo:phi, kvlo : kvlo + 128],
                qT[t][plo:phi, lo : lo + 512],
                start=True,
                stop=True,
                tile_position=tp,
            )
        return pS

    for t in range(IT):
        h0 = 2 * t
        kT_t = kT_tiles[t]
        if t + 1 < IT:
            cur_wk[t + 1] = load_wk(t + 1, nc.gpsimd)
        if t + 2 < IT:
            cur_wq[t + 2] = load_wq(t + 2, nc.gpsimd)
        wq_queue = weave_queue(t)
        for half in range(2):
            lo = half * 512
            pO = {}
            for hi in range(2):
                pO[hi] = psO.tile([65, 512], f32, name="ps_o", tag="ps_o")
            # kv pairs: batch the two row-tiled score pairs together, then the
            # exps, then the pO matmuls -> half as many PE tiling-mode
            # switches.  The pO block of pair p is deferred until after pair
            # p+1's scores are issued (software pipeline), so the PE always
            # has score/weave work in flight while ACT computes pair p's exps
            # -- this matters most at t>=6 where the weave queue is empty.
            def emit_pO(kvs, Es):
                for kv, E in zip(kvs, Es):
                    for hi in range(2):
                        nc.tensor.matmul(
                            pO[hi],
                            v_sb[kv][:, (h0 + hi) * 65 : (h0 + hi) * 65 + 65],
                            E[:, hi, :],
                            start=(kv == 0),
                            stop=(kv == KVT - 1),
                        )

            prev = None
            for kvp in range(KVT // 2):
                kvs = (2 * kvp, 2 * kvp + 1)
                if t == 0 and half == 0:
                    for kv in kvs:
                        emit_C_chain(kv, 0)
                pSs = [emit_scores(t, half, kv, kT_t) for kv in kvs]
                Es = []
                for kv, pS in zip(kvs, pSs):
                    E = Ep.tile([128, 2, 512], bf16, name="E", tag="E")
                    nc.scalar.activation(
                        E.rearrange("p h c -> p (h c)"),
                        pS.rearrange("p h c -> p (h c)"),
                        FT.Exp,
                        scale=SCALE,
                    )
                    Es.append(E)
                if prev is not None:
                    emit_pO(*prev)
                prev = (kvs, Es)
                if wq_queue:
                    wq_queue.pop(0)()
            emit_pO(*prev)

            # normalization: copy psum out, PE-broadcast the ones-column row,
            # approx-reciprocal (1 instr vs ~6 cyc/elem iterative), scale.
            oc = normp.tile([65, 2, 512], f32r, name="oc", tag="oc")
            for hi in range(2):
                nc.vector.tensor_copy(oc[:, hi, :], pO[hi])
            prb = {}
            for hi in range(2):
                prb[hi] = psO.tile([64, 512], f32, name="ps_o", tag="ps_o")
                nc.tensor.matmul(
                    prb[hi],
                    ones64[64:65, :],
                    oc[64:65, hi, :],
                    start=True,
                    stop=True,
                )
            rb = normp.tile([64, 2, 512], f32, name="rb", tag="rb")
            for hi in range(2):
                nc.vector.reciprocal_approx_fast(rb[:, hi, :], prb[hi])
            for hi in range(2):
                dst = OT[t][hi * 64 : hi * 64 + 64, lo : lo + 512]
                nc.vector.tensor_tensor(
                    dst, oc[0:64, hi, :], rb[:, hi, :], op=AluOpType.mult
                )
        while wq_queue:
            wq_queue.pop(0)()

    # SBUF LIFO: normp, Ep, kTp, qryTp, wqkp, wfp, vp, ctxTp, qTp
    normp.release()
    Ep.release()
    kTp.release()
    qryTp.release()
    wqkp.release()
    wfp.release()
    vp.release()
    ctxTp.release()
    qTp.release()
    # PSUM LIFO: psO, psS, psK, wvx
    psO.release()
    psS.release()
    psK.release()
    wvx.release()

    # ---- Phase E: out = O @ w_out + b_out ----
    with (
        tc.tile_pool(name="osb", bufs=3) as osbp,
        tc.tile_pool(name="psE", bufs=4, space="PSUM") as psE,
    ):
        for m in range(NQT):
            o_sb = osbp.tile([128, QD], f32, name="osb", tag="osb")
            for half in range(2):
                lo = half * 512
                ps = psE.tile([128, 512], f32, name="ps_e", tag="ps_e")
                for i in range(IT):
                    nc.tensor.matmul(
                        ps,
                        OT[i][:, m * 128 : (m + 1) * 128],
                        wo[i][:, lo : lo + 512],
                        start=(i == 0),
                        stop=(i == IT - 1),
                    )
                nc.vector.tensor_tensor(
                    o_sb[:, lo : lo + 512],
                    ps,
                    bias_bc[:, lo : lo + 512],
                    op=AluOpType.add,
                )
            nc.gpsimd.dma_start(out[m * 128 : (m + 1) * 128, :], o_sb)

    wvop.release()
    OTp.release()
    const.release()


def build(reps=1):
    nc = bacc.Bacc("TRN2", target_bir_lowering=False, debug=False)
    T = declare(nc)
    with tile.TileContext(nc) as tc:
        for _ in range(reps):
            emit(nc, tc, T)
    nc.compile()
    return nc


_nc_cache = None


def _get_nc():
    global _nc_cache
    if _nc_cache is None:
        _nc_cache = build()
    return _nc_cache


def kernel(query, context, w_q, w_kv, w_out, b_out, **run_kwargs):
    nc = _get_nc()
    query = np.ascontiguousarray(query, dtype=np.float32)
    context = np.ascontiguousarray(context, dtype=np.float32)
    shared = {
        "w_q": np.ascontiguousarray(w_q, dtype=np.float32),
        "w_kv": np.ascontiguousarray(w_kv, dtype=np.float32),
        "w_out": np.ascontiguousarray(w_out, dtype=np.float32),
        "b_out": np.ascontiguousarray(b_out, dtype=np.float32),
    }
    in_maps = [
        {"query": query[b], "context": context[b], **shared} for b in range(B)
    ]
    res = run_bass_kernel_spmd(nc, in_maps, core_ids=list(range(B)), **run_kwargs)
    out = np.stack([res.results[b]["out"] for b in range(B)])
    if run_kwargs:
        kernel.last_result = res
    return out



# revision 22
# speedup vs baseline: 1.3186x; 1.0080x over previous
"""Cross-attention kernel v8 for Trainium2 (Bass/Tile), data-parallel over batch.

Per core: query [1024,1024], context [2048,768] -> out [1024,1024].

Changes vs v4 (290us -> 227us, back-to-back same-device runs):
  - attention kv-pair loop is software-pipelined: pair p's attn@v matmuls
    are emitted after pair p+1's score matmuls, so the in-order PE always
    has score/weave work covering the ACT exp latency.  Biggest effect at
    t>=6 where the weave queue of projection chains has drained.

Changes vs v2 (346us -> 214us measured):
  - kv tiles processed in PAIRS in the attention loop: the two row-tiled
    score matmul pairs are batched together, then the two exps, then the
    four attn@v matmuls -> half as many PE tiling-mode transitions.
  - softmax normalization uses nc.vector.reciprocal_approx_fast (single
    custom-DVE instruction, ~51 ULP) instead of nc.vector.reciprocal,
    which on real HW is an iterative ~6 cycle/element op that put ~100us+
    of DVE time on the critical path.
  - all matmuls bf16; weights converted once; exp on [128,1024] ACT tiles.
  - PSUM: psS 2x[128,2x512] (4 banks) + psO [65,512]x2 (2) + psK (1) +
    weave (1) = 8 banks.  (Matmul psum outputs must stay within one 2KB
    bank -> all matmul N<=512 fp32.)
  - B (q-proj), C (v-proj), kproj and E (out-proj) chains woven into the
    attention loop through the spare weave bank.

Measured-HW notes (don't regress these):
  - DVE partition-SHIFTED reads work for standard ops but silently produce
    garbage for custom-DVE ops (reciprocal_approx_*).  Keep custom-op APs
    at their natural base partition.
  - Adding work to the scores->exp->attn@v dependency chain on DVE/Pool
    (e.g. Schraudolph exp on DVE, f32->bf16 converts feeding transposes)
    regressed wall time by ~40%: those engines' per-instruction latency is
    far higher than the cost model suggests.  Keep ACT as the only exp
    engine and PE fed straight from DMA'd inputs.
"""

import numpy as np

import concourse.bass as bass
import concourse.tile as tile
from concourse import bacc, mybir
from concourse.alu_op_type import AluOpType
from concourse.bass_utils import run_bass_kernel_spmd
from concourse.masks import make_identity

NQ, QD, CD, NKV = 1024, 1024, 768, 2048
H, DH, INNER = 16, 64, 1024
SCALE = DH**-0.5
NQT, QDT, CDT, KVT, IT = NQ // 128, QD // 128, CD // 128, NKV // 128, INNER // 128
B = 8

f32 = mybir.dt.float32
f32r = mybir.dt.float32r
bf16 = mybir.dt.bfloat16
i16 = mybir.dt.int16
FT = mybir.ActivationFunctionType

# Schraudolph exp on DVE for a subset of kv tiles (offloads the ACT engine).
# E_bits(bf16) = round(128*(s*SCALE*log2(e) + 127 - sigma)); the constant
# sigma / rounding-mode bias is a pure scale factor on exp and cancels in the
# softmax normalization.
SCH_A = 128.0 * SCALE * 1.4426950408889634
SCH_B = 128.0 * (127.0 - 0.0430)
DVE_EXP_EVERY = 0  # kv % N == N-1 tiles go to DVE; 0 disables
BF16_TRANSPOSE = False  # phase-A transposes in bf16 (regressed on HW: the
# f32->bf16 convert copies put Pool/ACT latency on the transpose dep chain)


def declare(nc):
    return dict(
        query=nc.dram_tensor("query", [NQ, QD], f32, kind="ExternalInput"),
        context=nc.dram_tensor("context", [NKV, CD], f32, kind="ExternalInput"),
        w_q=nc.dram_tensor("w_q", [QD, INNER], f32, kind="ExternalInput"),
        w_kv=nc.dram_tensor("w_kv", [CD, 2 * INNER], f32, kind="ExternalInput"),
        w_out=nc.dram_tensor("w_out", [INNER, QD], f32, kind="ExternalInput"),
        b_out=nc.dram_tensor("b_out", [QD], f32, kind="ExternalInput"),
        out=nc.dram_tensor("out", [NQ, QD], f32, kind="ExternalOutput"),
    )


def emit(nc, tc, T):
    query, context, w_q, w_kv = T["query"], T["context"], T["w_q"], T["w_kv"]
    w_out, b_out, out = T["w_out"], T["b_out"], T["out"]

    const = tc.alloc_tile_pool(name="const", bufs=1)
    ident_f = const.tile([128, 128], f32, name="ident_f", tag="ident_f")
    make_identity(nc, ident_f)
    ident = const.tile([128, 128], f32r, name="ident", tag="ident")
    nc.vector.tensor_copy(ident, ident_f)
    if BF16_TRANSPOSE:
        ident_b = const.tile([128, 128], bf16, name="ident_b", tag="ident_b")
        nc.vector.tensor_copy(ident_b, ident_f)
    ones64_f = const.tile([128, 64], f32, name="ones64_f", tag="ones64_f")
    nc.vector.memset(ones64_f, 1.0)
    ones64 = const.tile([128, 64], f32r, name="ones64", tag="ones64")
    nc.vector.tensor_copy(ones64, ones64_f)
    bias_bc = const.tile([128, QD], f32, name="bias", tag="bias")
    nc.sync.dma_start(bias_bc, b_out[:].partition_broadcast(128))

    # ---- stack order: const, OTp, wvop survive into phase E; the rest
    # (qTp..normp) are released LIFO before it. ----
    OTp = tc.alloc_tile_pool(name="OTp", bufs=1)
    OT = [OTp.tile([128, NQ], bf16, name=f"OT{t}", tag=f"OT{t}") for t in range(IT)]
    wvop = tc.alloc_tile_pool(name="wvop", bufs=1)
    wv = [
        wvop.tile([128, INNER], bf16, name=f"wv{j}", tag=f"wv{j}")
        for j in range(CDT)
    ]
    wo = [wvop.tile([128, QD], bf16, name=f"wo{i}", tag=f"wo{i}") for i in range(IT)]

    qTp = tc.alloc_tile_pool(name="qTp", bufs=1)
    qT = [qTp.tile([128, NQ], bf16, name=f"qT{m}", tag=f"qT{m}") for m in range(IT)]
    ctxTp = tc.alloc_tile_pool(name="ctxTp", bufs=1)
    ctxT = [
        ctxTp.tile([128, NKV], bf16, name=f"ctxT{j}", tag=f"ctxT{j}")
        for j in range(CDT)
    ]
    vp = tc.alloc_tile_pool(name="vp", bufs=1)
    v_sb = [
        vp.tile([128, H * 65], bf16, name=f"v{t}", tag=f"v{t}") for t in range(KVT)
    ]
    wfp = tc.alloc_tile_pool(name="wfp", bufs=3)
    wqkp = tc.alloc_tile_pool(name="wqkp", bufs=2)

    def stage(shape, src_ap, dst, eng):
        s = wfp.tile([128, 1024], f32, name="wst", tag="wst")
        sv = s[:, : shape[1] * shape[2]].rearrange(
            "p (a b) -> p a b", a=shape[1]
        ) if len(shape) == 3 else s[:, : shape[1]]
        nc.sync.dma_start(sv, src_ap)
        eng.tensor_copy(dst, sv)

    def load_wv(j, eng):
        stage(
            [128, INNER],
            w_kv[j * 128 : (j + 1) * 128, INNER : 2 * INNER],
            wv[j],
            eng,
        )

    def load_wo(i, eng):
        stage([128, QD], w_out[i * 128 : (i + 1) * 128, :], wo[i], eng)

    def load_wq(m, eng):
        t = wqkp.tile([128, QDT, 128], bf16, name="wqb", tag="wqb")
        src = bass.AP(
            tensor=w_q,
            offset=m * 128,
            ap=[[INNER, 128], [128 * INNER, QDT], [1, 128]],
        )
        stage([128, QDT, 128], src, t, eng)
        return t

    def load_wk(t_, eng):
        t = wqkp.tile([128, CDT, 128], bf16, name="wkb", tag="wkb")
        src = bass.AP(
            tensor=w_kv,
            offset=t_ * 128,
            ap=[[2 * INNER, 128], [128 * 2 * INNER, CDT], [1, 128]],
        )
        stage([128, CDT, 128], src, t, eng)
        return t

    # ---- Phase A: load & transpose query and context ----
    qryTp = tc.alloc_tile_pool(name="qryTp", bufs=1)
    queryT = [
        qryTp.tile([128, NQ], bf16, name=f"qryT{j}", tag=f"qryT{j}")
        for j in range(QDT)
    ]
    cur_wq, cur_wk = {}, {}
    with (
        tc.tile_pool(name="phA", bufs=2) as phA,
        tc.tile_pool(name="psA", bufs=8, space="PSUM") as psA,
    ):
        engs = [nc.vector.tensor_copy, nc.scalar.copy]
        cvt = [nc.scalar.copy, nc.gpsimd.tensor_copy]
        cur_wk[0] = load_wk(0, nc.vector)
        for i in range(KVT):
            if BF16_TRANSPOSE:
                cnat = phA.tile([128, CD], f32, name="cnat", tag="cnat")
                nc.sync.dma_start(cnat, context[i * 128 : (i + 1) * 128, :])
                cnb = phA.tile([128, CD], bf16, name="cnb", tag="cnb")
                cvt[i % 2](cnb, cnat)
                for j in range(CDT):
                    pt = psA.tile([128, 128], bf16, name="ptb", tag="ptb")
                    nc.tensor.transpose(pt, cnb[:, j * 128 : (j + 1) * 128], ident_b)
                    engs[(i + j) % 2](ctxT[j][:, i * 128 : (i + 1) * 128], pt)
            else:
                cnat = phA.tile([128, CD], f32r, name="cnat", tag="cnat")
                nc.sync.dma_start(
                    cnat, context[i * 128 : (i + 1) * 128, :].bitcast(f32r)
                )
                for j in range(CDT):
                    pt = psA.tile([128, 128], f32r, name="pt", tag="pt")
                    nc.tensor.transpose(pt, cnat[:, j * 128 : (j + 1) * 128], ident)
                    engs[(i + j) % 2](ctxT[j][:, i * 128 : (i + 1) * 128], pt)
        for j in range(CDT):
            load_wv(j, (nc.vector, nc.gpsimd)[j % 2])
        for i in range(NQT):
            if BF16_TRANSPOSE:
                qnat = phA.tile([128, QD], f32, name="qnat", tag="qnat")
                nc.sync.dma_start(qnat, query[i * 128 : (i + 1) * 128, :])
                qnb = phA.tile([128, QD], bf16, name="qnb", tag="qnb")
                cvt[i % 2](qnb, qnat)
                for j in range(QDT):
                    pt = psA.tile([128, 128], bf16, name="ptb", tag="ptb")
                    nc.tensor.transpose(pt, qnb[:, j * 128 : (j + 1) * 128], ident_b)
                    engs[(i + j) % 2](queryT[j][:, i * 128 : (i + 1) * 128], pt)
            else:
                qnat = phA.tile([128, QD], f32r, name="qnat", tag="qnat")
                nc.sync.dma_start(
                    qnat, query[i * 128 : (i + 1) * 128, :].bitcast(f32r)
                )
                for j in range(QDT):
                    pt = psA.tile([128, 128], f32r, name="pt", tag="pt")
                    nc.tensor.transpose(pt, qnat[:, j * 128 : (j + 1) * 128], ident)
                    engs[(i + j) % 2](queryT[j][:, i * 128 : (i + 1) * 128], pt)
        for i in range(IT):
            load_wo(i, (nc.vector, nc.gpsimd)[i % 2])
        cur_wq[0] = load_wq(0, nc.vector)
        cur_wq[1] = load_wq(1, nc.gpsimd)

    # ---- weave machinery ----
    wvx = tc.alloc_tile_pool(name="wvx", bufs=1, space="PSUM")
    psK = tc.alloc_tile_pool(name="psK", bufs=1, space="PSUM")
    kTp = tc.alloc_tile_pool(name="kTp", bufs=2)

    def emit_B_half(m, h):
        ps = wvx.tile([128, 512], f32, name="ps_w", tag="ps_w")
        for j in range(QDT):
            nc.tensor.matmul(
                ps,
                cur_wq[m][:, j, :],
                queryT[j][:, h * 512 : (h + 1) * 512],
                start=(j == 0),
                stop=(j == QDT - 1),
            )
        nc.vector.tensor_copy(qT[m][:, h * 512 : (h + 1) * 512], ps)

    def emit_C_chain(kv, h):
        ps = wvx.tile([128, 512], f32, name="ps_w", tag="ps_w")
        for j in range(CDT):
            nc.tensor.matmul(
                ps,
                ctxT[j][:, kv * 128 : (kv + 1) * 128],
                wv[j][:, h * 512 : (h + 1) * 512],
                start=(j == 0),
                stop=(j == CDT - 1),
            )
        vt = v_sb[kv].rearrange("p (h c) -> p h c", c=65)
        nc.vector.tensor_copy(
            vt[:, h * 8 : (h + 1) * 8, 0:64],
            ps.rearrange("p (h c) -> p h c", c=64),
        )
        nc.vector.memset(vt[:, h * 8 : (h + 1) * 8, 64:65], 1.0)

    def emit_kproj_chain(t, kT_t, n):
        ps = psK.tile([128, 512], f32, name="ps_k", tag="ps_k")
        for j in range(CDT):
            nc.tensor.matmul(
                ps,
                cur_wk[t][:, j, :],
                ctxT[j][:, n * 512 : (n + 1) * 512],
                start=(j == 0),
                stop=(j == CDT - 1),
            )
        nc.vector.tensor_copy(kT_t[:, n * 512 : (n + 1) * 512], ps)

    kT_tiles = {}

    def make_kT(t):
        kT_tiles[t] = kTp.tile([128, NKV], bf16, name=f"kT{t}", tag="kT")
        return kT_tiles[t]

    # upfront: qT[0], kT[0]; B(t+1) is woven during t so the late head-pairs
    # (t=6) still have chains covering the exp latency
    kT0 = make_kT(0)
    for h in range(2):
        emit_B_half(0, h)
    for n in range(4):
        emit_kproj_chain(0, kT0, n)

    def weave_queue(t):
        q = []
        if t < IT - 1:
            kt = make_kT(t + 1)
            for n in range(4):
                q.append(lambda n=n, t=t, kt=kt: emit_kproj_chain(t + 1, kt, n))
        if t + 1 < IT:
            for h in range(2):
                q.append(lambda h=h, t=t: emit_B_half(t + 1, h))
        if t < 4:
            for kv in range(4 * t, 4 * t + 4):
                q.append(lambda kv=kv: emit_C_chain(kv, 1))
        return q

    # ---- Phase D ----
    Ep = tc.alloc_tile_pool(name="Ep", bufs=4)
    normp = tc.alloc_tile_pool(name="norm", bufs=2)
    psS = tc.alloc_tile_pool(name="psS", bufs=2, space="PSUM")
    psO = tc.alloc_tile_pool(name="psO", bufs=2, space="PSUM")

    def emit_scores(t, half, kv, kT_t):
        lo = half * 512
        kvlo = kv * 128
        pS = psS.tile([128, 2, 512], f32, name="ps_s", tag="ps_s")
        for hi in range(2):
            plo, phi = (0, 64) if hi == 0 else (64, 128)
            tp = (0, 0) if hi == 0 else (64, 0)
            nc.tensor.matmul(
                pS[:, hi, :],
                kT_t[plo:phi, kvlo : kvlo + 128],
                qT[t][plo:phi, lo : lo + 512],
                start=True,
                stop=True,
                tile_position=tp,
            )
        return pS

    for t in range(IT):
        h0 = 2 * t
        kT_t = kT_tiles[t]
        if t + 1 < IT:
            cur_wk[t + 1] = load_wk(t + 1, nc.gpsimd)
        if t + 2 < IT:
            cur_wq[t + 2] = load_wq(t + 2, nc.gpsimd)
        wq_queue = weave_queue(t)
        for half in range(2):
            lo = half * 512
            pO = {}
            for hi in range(2):
                pO[hi] = psO.tile([65, 512], f32, name="ps_o", tag="ps_o")
            # kv pairs: batch the two row-tiled score pairs together, then the
            # exps, then the pO matmuls -> half as many PE tiling-mode
            # switches.  The pO block of pair p is deferred until after pair
            # p+1's scores are issued (software pipeline), so the PE always
            # has score/weave work in flight while ACT computes pair p's exps
            # -- this matters most at t>=6 where the weave queue is empty.
            def emit_pO(kvs, Es):
                for kv, E in zip(kvs, Es):
                    for hi in range(2):
                        nc.tensor.matmul(
                            pO[hi],
                            v_sb[kv][:, (h0 + hi) * 65 : (h0 + hi) * 65 + 65],
                            E[:, hi, :],
                            start=(kv == 0),
                            stop=(kv == KVT - 1),
                        )

            prev = None
            for kvp in range(KVT // 2):
                kvs = (2 * kvp, 2 * kvp + 1)
                if t == 0 and half == 0:
                    for kv in kvs:
                        emit_C_chain(kv, 0)
                pSs = [emit_scores(t, half, kv, kT_t) for kv in kvs]
                Es = []
                for kv, pS in zip(kvs, pSs):
                    if t >= 6 and kv % 2 == 1:
                        # late head-pairs have no weave chains left; pairs run
                        # at ACT cadence (2 serial exps).  Offload the second
                        # exp to the (idle-there) DVE via the Schraudolph
                        # bf16-bits trick so the two exps run in parallel.
                        Ei = Ep.tile([128, 2, 512], i16, name="E", tag="E")
                        nc.vector.tensor_scalar(
                            Ei.rearrange("p h c -> p (h c)"),
                            pS.rearrange("p h c -> p (h c)"),
                            SCH_A,
                            SCH_B,
                            op0=AluOpType.mult,
                            op1=AluOpType.add,
                        )
                        E = Ei.bitcast(bf16)
                    else:
                        E = Ep.tile([128, 2, 512], bf16, name="E", tag="E")
                        nc.scalar.activation(
                            E.rearrange("p h c -> p (h c)"),
                            pS.rearrange("p h c -> p (h c)"),
                            FT.Exp,
                            scale=SCALE,
                        )
                    Es.append(E)
                if prev is not None:
                    emit_pO(*prev)
                prev = (kvs, Es)
                if wq_queue:
                    wq_queue.pop(0)()
            emit_pO(*prev)

            # normalization: copy psum out, PE-broadcast the ones-column row,
            # approx-reciprocal (1 instr vs ~6 cyc/elem iterative), scale.
            oc = normp.tile([65, 2, 512], f32r, name="oc", tag="oc")
            for hi in range(2):
                nc.vector.tensor_copy(oc[:, hi, :], pO[hi])
            prb = {}
            for hi in range(2):
                prb[hi] = psO.tile([64, 512], f32, name="ps_o", tag="ps_o")
                nc.tensor.matmul(
                    prb[hi],
                    ones64[64:65, :],
                    oc[64:65, hi, :],
                    start=True,
                    stop=True,
                )
            rb = normp.tile([64, 2, 512], f32, name="rb", tag="rb")
            for hi in range(2):
                nc.vector.reciprocal_approx_fast(rb[:, hi, :], prb[hi])
            for hi in range(2):
                dst = OT[t][hi * 64 : hi * 64 + 64, lo : lo + 512]
                nc.vector.tensor_tensor(
                    dst, oc[0:64, hi, :], rb[:, hi, :], op=AluOpType.mult
                )
        while wq_queue:
            wq_queue.pop(0)()

    # SBUF LIFO: normp, Ep, kTp, qryTp, wqkp, wfp, vp, ctxTp, qTp
    normp.release()
    Ep.release()
    kTp.release()
    qryTp.release()
    wqkp.release()
    wfp.release()
    vp.release()
    ctxTp.release()
    qTp.release()
    # PSUM LIFO: psO, psS, psK, wvx
    psO.release()
    psS.release()
    psK.release()
    wvx.release()

    # ---- Phase E: out = O @ w_out + b_out ----
    with (
        tc.tile_pool(name="osb", bufs=3) as osbp,
        tc.tile_pool(name="psE", bufs=4, space="PSUM") as psE,
    ):
        for m in range(NQT):
            o_sb = osbp.tile([128, QD], f32, name="osb", tag="osb")
            for half in range(2):
                lo = half * 512
                ps = psE.tile([128, 512], f32, name="ps_e", tag="ps_e")
                for i in range(IT):
                    nc.tensor.matmul(
                        ps,
                        OT[i][:, m * 128 : (m + 1) * 128],
                        wo[i][:, lo : lo + 512],
                        start=(i == 0),
                        stop=(i == IT - 1),
                    )
                nc.vector.tensor_tensor(
                    o_sb[:, lo : lo + 512],
                    ps,
                    bias_bc[:, lo : lo + 512],
                    op=AluOpType.add,
                )
            nc.gpsimd.dma_start(out[m * 128 : (m + 1) * 128, :], o_sb)

    wvop.release()
    OTp.release()
    const.release()


def build(reps=1):
    nc = bacc.Bacc("TRN2", target_bir_lowering=False, debug=False)
    T = declare(nc)
    with tile.TileContext(nc) as tc:
        for _ in range(reps):
            emit(nc, tc, T)
    nc.compile()
    return nc


_nc_cache = None


def _get_nc():
    global _nc_cache
    if _nc_cache is None:
        _nc_cache = build()
    return _nc_cache


def kernel(query, context, w_q, w_kv, w_out, b_out, **run_kwargs):
    nc = _get_nc()
    query = np.ascontiguousarray(query, dtype=np.float32)
    context = np.ascontiguousarray(context, dtype=np.float32)
    shared = {
        "w_q": np.ascontiguousarray(w_q, dtype=np.float32),
        "w_kv": np.ascontiguousarray(w_kv, dtype=np.float32),
        "w_out": np.ascontiguousarray(w_out, dtype=np.float32),
        "b_out": np.ascontiguousarray(b_out, dtype=np.float32),
    }
    in_maps = [
        {"query": query[b], "context": context[b], **shared} for b in range(B)
    ]
    res = run_bass_kernel_spmd(nc, in_maps, core_ids=list(range(B)), **run_kwargs)
    out = np.stack([res.results[b]["out"] for b in range(B)])
    if run_kwargs:
        kernel.last_result = res
    return out



# revision 25
# speedup vs baseline: 1.5189x; 1.1519x over previous
"""Cross-attention kernel v9 for Trainium2 (Bass/Tile), data-parallel over batch.

Per core: query [1024,1024], context [2048,768] -> out [1024,1024].
Measured: 225.4us, rel err 6.0e-3 (vs 346us / 4.7e-3 baseline).

Changes vs v8 (227.3us -> 225.4us, rel err 4.7e-3 -> 6.0e-3):
  - at t>=6 (weave queue empty, pairs run at ACT cadence) the second exp of
    each kv pair runs on the otherwise-idle DVE via the Schraudolph
    bf16-bits trick (12.5% of exps; the constant-factor bias cancels in the
    softmax normalization).
  - B (q-proj) chains woven at t+1 instead of t+2 so t=6 keeps chain cover.

Changes vs v4 (290us -> 227us, back-to-back same-device runs):
  - attention kv-pair loop is software-pipelined: pair p's attn@v matmuls
    are emitted after pair p+1's score matmuls, so the in-order PE always
    has score/weave work covering the ACT exp latency.  Biggest effect at
    t>=6 where the weave queue of projection chains has drained.

Changes vs v2 (346us -> 214us measured):
  - kv tiles processed in PAIRS in the attention loop: the two row-tiled
    score matmul pairs are batched together, then the two exps, then the
    four attn@v matmuls -> half as many PE tiling-mode transitions.
  - softmax normalization uses nc.vector.reciprocal_approx_fast (single
    custom-DVE instruction, ~51 ULP) instead of nc.vector.reciprocal,
    which on real HW is an iterative ~6 cycle/element op that put ~100us+
    of DVE time on the critical path.
  - all matmuls bf16; weights converted once; exp on [128,1024] ACT tiles.
  - PSUM: psS 2x[128,2x512] (4 banks) + psO [65,512]x2 (2) + psK (1) +
    weave (1) = 8 banks.  (Matmul psum outputs must stay within one 2KB
    bank -> all matmul N<=512 fp32.)
  - B (q-proj), C (v-proj), kproj and E (out-proj) chains woven into the
    attention loop through the spare weave bank.

Measured-HW notes (don't regress these):
  - DVE partition-SHIFTED reads work for standard ops but silently produce
    garbage for custom-DVE ops (reciprocal_approx_*).  Keep custom-op APs
    at their natural base partition.
  - Adding work to the scores->exp->attn@v dependency chain on DVE/Pool
    (e.g. Schraudolph exp on DVE, f32->bf16 converts feeding transposes)
    regressed wall time by ~40%: those engines' per-instruction latency is
    far higher than the cost model suggests.  Keep ACT as the only exp
    engine and PE fed straight from DMA'd inputs.
"""

import numpy as np

import concourse.bass as bass
import concourse.tile as tile
from concourse import bacc, mybir
from concourse.alu_op_type import AluOpType
from concourse.bass_utils import run_bass_kernel_spmd
from concourse.masks import make_identity

NQ, QD, CD, NKV = 1024, 1024, 768, 2048
H, DH, INNER = 16, 64, 1024
SCALE = DH**-0.5
NQT, QDT, CDT, KVT, IT = NQ // 128, QD // 128, CD // 128, NKV // 128, INNER // 128
B = 8

f32 = mybir.dt.float32
f32r = mybir.dt.float32r
bf16 = mybir.dt.bfloat16
i16 = mybir.dt.int16
FT = mybir.ActivationFunctionType

# Schraudolph exp on DVE for a subset of kv tiles (offloads the ACT engine).
# E_bits(bf16) = round(128*(s*SCALE*log2(e) + 127 - sigma)); the constant
# sigma / rounding-mode bias is a pure scale factor on exp and cancels in the
# softmax normalization.
SCH_A = 128.0 * SCALE * 1.4426950408889634
SCH_B = 128.0 * (127.0 - 0.0430)
DVE_EXP_EVERY = 0  # kv % N == N-1 tiles go to DVE; 0 disables
BF16_TRANSPOSE = False  # phase-A transposes in bf16 (regressed on HW: the
# f32->bf16 convert copies put Pool/ACT latency on the transpose dep chain)


def declare(nc):
    return dict(
        query=nc.dram_tensor("query", [NQ, QD], f32, kind="ExternalInput"),
        context=nc.dram_tensor("context", [NKV, CD], f32, kind="ExternalInput"),
        w_q=nc.dram_tensor("w_q", [QD, INNER], f32, kind="ExternalInput"),
        w_kv=nc.dram_tensor("w_kv", [CD, 2 * INNER], f32, kind="ExternalInput"),
        w_out=nc.dram_tensor("w_out", [INNER, QD], f32, kind="ExternalInput"),
        b_out=nc.dram_tensor("b_out", [QD], f32, kind="ExternalInput"),
        out=nc.dram_tensor("out", [NQ, QD], f32, kind="ExternalOutput"),
    )


def emit(nc, tc, T):
    query, context, w_q, w_kv = T["query"], T["context"], T["w_q"], T["w_kv"]
    w_out, b_out, out = T["w_out"], T["b_out"], T["out"]

    const = tc.alloc_tile_pool(name="const", bufs=1)
    ident_f = const.tile([128, 128], f32, name="ident_f", tag="ident_f")
    make_identity(nc, ident_f)
    ident = const.tile([128, 128], f32r, name="ident", tag="ident")
    nc.vector.tensor_copy(ident, ident_f)
    if BF16_TRANSPOSE:
        ident_b = const.tile([128, 128], bf16, name="ident_b", tag="ident_b")
        nc.vector.tensor_copy(ident_b, ident_f)
    ones64_f = const.tile([128, 64], f32, name="ones64_f", tag="ones64_f")
    nc.vector.memset(ones64_f, 1.0)
    ones64 = const.tile([128, 64], f32r, name="ones64", tag="ones64")
    nc.vector.tensor_copy(ones64, ones64_f)
    bias_bc = const.tile([128, QD], f32, name="bias", tag="bias")
    nc.sync.dma_start(bias_bc, b_out[:].partition_broadcast(128))

    # ---- stack order: const, OTp, wvop survive into phase E; the rest
    # (qTp..normp) are released LIFO before it. ----
    OTp = tc.alloc_tile_pool(name="OTp", bufs=1)
    OT = [OTp.tile([128, NQ], bf16, name=f"OT{t}", tag=f"OT{t}") for t in range(IT)]
    wvop = tc.alloc_tile_pool(name="wvop", bufs=1)
    wv = [
        wvop.tile([128, INNER], bf16, name=f"wv{j}", tag=f"wv{j}")
        for j in range(CDT)
    ]
    wo = [wvop.tile([128, QD], bf16, name=f"wo{i}", tag=f"wo{i}") for i in range(IT)]

    qTp = tc.alloc_tile_pool(name="qTp", bufs=1)
    qT = [qTp.tile([128, NQ], bf16, name=f"qT{m}", tag=f"qT{m}") for m in range(IT)]
    ctxTp = tc.alloc_tile_pool(name="ctxTp", bufs=1)
    ctxT = [
        ctxTp.tile([128, NKV], bf16, name=f"ctxT{j}", tag=f"ctxT{j}")
        for j in range(CDT)
    ]
    vp = tc.alloc_tile_pool(name="vp", bufs=1)
    v_sb = [
        vp.tile([128, H * 65], bf16, name=f"v{t}", tag=f"v{t}") for t in range(KVT)
    ]
    wfp = tc.alloc_tile_pool(name="wfp", bufs=3)
    wqkp = tc.alloc_tile_pool(name="wqkp", bufs=2)

    def stage(shape, src_ap, dst, eng):
        s = wfp.tile([128, 1024], f32, name="wst", tag="wst")
        sv = s[:, : shape[1] * shape[2]].rearrange(
            "p (a b) -> p a b", a=shape[1]
        ) if len(shape) == 3 else s[:, : shape[1]]
        nc.sync.dma_start(sv, src_ap)
        eng.tensor_copy(dst, sv)

    def load_wv(j, eng):
        stage(
            [128, INNER],
            w_kv[j * 128 : (j + 1) * 128, INNER : 2 * INNER],
            wv[j],
            eng,
        )

    def load_wo(i, eng):
        stage([128, QD], w_out[i * 128 : (i + 1) * 128, :], wo[i], eng)

    def load_wq(m, eng):
        t = wqkp.tile([128, QDT, 128], bf16, name="wqb", tag="wqb")
        src = bass.AP(
            tensor=w_q,
            offset=m * 128,
            ap=[[INNER, 128], [128 * INNER, QDT], [1, 128]],
        )
        stage([128, QDT, 128], src, t, eng)
        return t

    def load_wk(t_, eng):
        t = wqkp.tile([128, CDT, 128], bf16, name="wkb", tag="wkb")
        src = bass.AP(
            tensor=w_kv,
            offset=t_ * 128,
            ap=[[2 * INNER, 128], [128 * 2 * INNER, CDT], [1, 128]],
        )
        stage([128, CDT, 128], src, t, eng)
        return t

    # ---- Phase A: load & transpose query and context ----
    qryTp = tc.alloc_tile_pool(name="qryTp", bufs=1)
    queryT = [
        qryTp.tile([128, NQ], bf16, name=f"qryT{j}", tag=f"qryT{j}")
        for j in range(QDT)
    ]
    cur_wq, cur_wk = {}, {}
    with (
        tc.tile_pool(name="phA", bufs=2) as phA,
        tc.tile_pool(name="psA", bufs=8, space="PSUM") as psA,
    ):
        engs = [nc.vector.tensor_copy, nc.scalar.copy]
        cvt = [nc.scalar.copy, nc.gpsimd.tensor_copy]
        cur_wk[0] = load_wk(0, nc.vector)
        for i in range(KVT):
            if BF16_TRANSPOSE:
                cnat = phA.tile([128, CD], f32, name="cnat", tag="cnat")
                nc.sync.dma_start(cnat, context[i * 128 : (i + 1) * 128, :])
                cnb = phA.tile([128, CD], bf16, name="cnb", tag="cnb")
                cvt[i % 2](cnb, cnat)
                for j in range(CDT):
                    pt = psA.tile([128, 128], bf16, name="ptb", tag="ptb")
                    nc.tensor.transpose(pt, cnb[:, j * 128 : (j + 1) * 128], ident_b)
                    engs[(i + j) % 2](ctxT[j][:, i * 128 : (i + 1) * 128], pt)
            else:
                cnat = phA.tile([128, CD], f32r, name="cnat", tag="cnat")
                nc.sync.dma_start(
                    cnat, context[i * 128 : (i + 1) * 128, :].bitcast(f32r)
                )
                for j in range(CDT):
                    pt = psA.tile([128, 128], f32r, name="pt", tag="pt")
                    nc.tensor.transpose(pt, cnat[:, j * 128 : (j + 1) * 128], ident)
                    engs[(i + j) % 2](ctxT[j][:, i * 128 : (i + 1) * 128], pt)
        for j in range(CDT):
            load_wv(j, (nc.vector, nc.gpsimd)[j % 2])
        for i in range(NQT):
            if BF16_TRANSPOSE:
                qnat = phA.tile([128, QD], f32, name="qnat", tag="qnat")
                nc.sync.dma_start(qnat, query[i * 128 : (i + 1) * 128, :])
                qnb = phA.tile([128, QD], bf16, name="qnb", tag="qnb")
                cvt[i % 2](qnb, qnat)
                for j in range(QDT):
                    pt = psA.tile([128, 128], bf16, name="ptb", tag="ptb")
                    nc.tensor.transpose(pt, qnb[:, j * 128 : (j + 1) * 128], ident_b)
                    engs[(i + j) % 2](queryT[j][:, i * 128 : (i + 1) * 128], pt)
            else:
                qnat = phA.tile([128, QD], f32r, name="qnat", tag="qnat")
                nc.sync.dma_start(
                    qnat, query[i * 128 : (i + 1) * 128, :].bitcast(f32r)
                )
                for j in range(QDT):
                    pt = psA.tile([128, 128], f32r, name="pt", tag="pt")
                    nc.tensor.transpose(pt, qnat[:, j * 128 : (j + 1) * 128], ident)
                    engs[(i + j) % 2](queryT[j][:, i * 128 : (i + 1) * 128], pt)
        for i in range(IT):
            load_wo(i, (nc.vector, nc.gpsimd)[i % 2])
        cur_wq[0] = load_wq(0, nc.vector)
        cur_wq[1] = load_wq(1, nc.gpsimd)

    # ---- weave machinery ----
    wvx = tc.alloc_tile_pool(name="wvx", bufs=1, space="PSUM")
    psK = tc.alloc_tile_pool(name="psK", bufs=1, space="PSUM")
    kTp = tc.alloc_tile_pool(name="kTp", bufs=2)

    def emit_B_half(m, h):
        ps = wvx.tile([128, 512], f32, name="ps_w", tag="ps_w")
        for j in range(QDT):
            nc.tensor.matmul(
                ps,
                cur_wq[m][:, j, :],
                queryT[j][:, h * 512 : (h + 1) * 512],
                start=(j == 0),
                stop=(j == QDT - 1),
            )
        nc.vector.tensor_copy(qT[m][:, h * 512 : (h + 1) * 512], ps)

    def emit_C_chain(kv, h):
        ps = wvx.tile([128, 512], f32, name="ps_w", tag="ps_w")
        for j in range(CDT):
            nc.tensor.matmul(
                ps,
                ctxT[j][:, kv * 128 : (kv + 1) * 128],
                wv[j][:, h * 512 : (h + 1) * 512],
                start=(j == 0),
                stop=(j == CDT - 1),
            )
        vt = v_sb[kv].rearrange("p (h c) -> p h c", c=65)
        nc.vector.tensor_copy(
            vt[:, h * 8 : (h + 1) * 8, 0:64],
            ps.rearrange("p (h c) -> p h c", c=64),
        )
        nc.vector.memset(vt[:, h * 8 : (h + 1) * 8, 64:65], 1.0)

    def emit_kproj_chain(t, kT_t, n):
        ps = psK.tile([128, 512], f32, name="ps_k", tag="ps_k")
        for j in range(CDT):
            nc.tensor.matmul(
                ps,
                cur_wk[t][:, j, :],
                ctxT[j][:, n * 512 : (n + 1) * 512],
                start=(j == 0),
                stop=(j == CDT - 1),
            )
        nc.vector.tensor_copy(kT_t[:, n * 512 : (n + 1) * 512], ps)

    kT_tiles = {}

    def make_kT(t):
        kT_tiles[t] = kTp.tile([128, NKV], bf16, name=f"kT{t}", tag="kT")
        return kT_tiles[t]

    # upfront: qT[0], kT[0]; B(t+1) is woven during t so the late head-pairs
    # (t=6) still have chains covering the exp latency
    kT0 = make_kT(0)
    for h in range(2):
        emit_B_half(0, h)
    for n in range(4):
        emit_kproj_chain(0, kT0, n)

    def weave_queue(t):
        q = []
        if t < IT - 1:
            kt = make_kT(t + 1)
            for n in range(4):
                q.append(lambda n=n, t=t, kt=kt: emit_kproj_chain(t + 1, kt, n))
        if t + 1 < IT:
            for h in range(2):
                q.append(lambda h=h, t=t: emit_B_half(t + 1, h))
        # C(h=1) deadline is t=4; keep them out of the PE-heavy t=0 (which
        # already runs the 16 inline C(h=0) chains) and spread over t=1..3
        C_SLOTS = {1: range(0, 6), 2: range(6, 11), 3: range(11, 16)}
        if t in C_SLOTS:
            for kv in C_SLOTS[t]:
                q.append(lambda kv=kv: emit_C_chain(kv, 1))
        return q

    # ---- Phase D ----
    Ep = tc.alloc_tile_pool(name="Ep", bufs=4)
    normp = tc.alloc_tile_pool(name="norm", bufs=2)
    psS = tc.alloc_tile_pool(name="psS", bufs=2, space="PSUM")
    psO = tc.alloc_tile_pool(name="psO", bufs=2, space="PSUM")

    def emit_scores(t, half, kv, kT_t):
        lo = half * 512
        kvlo = kv * 128
        pS = psS.tile([128, 2, 512], f32, name="ps_s", tag="ps_s")
        for hi in range(2):
            plo, phi = (0, 64) if hi == 0 else (64, 128)
            tp = (0, 0) if hi == 0 else (64, 0)
            nc.tensor.matmul(
                pS[:, hi, :],
                kT_t[plo:phi, kvlo : kvlo + 128],
                qT[t][plo:phi, lo : lo + 512],
                start=True,
                stop=True,
                tile_position=tp,
            )
        return pS

    for t in range(IT):
        h0 = 2 * t
        kT_t = kT_tiles[t]
        if t + 1 < IT:
            cur_wk[t + 1] = load_wk(t + 1, nc.gpsimd)
        if t + 2 < IT:
            cur_wq[t + 2] = load_wq(t + 2, nc.gpsimd)
        wq_queue = weave_queue(t)
        for half in range(2):
            lo = half * 512
            pO = {}
            for hi in range(2):
                pO[hi] = psO.tile([65, 512], f32, name="ps_o", tag="ps_o")
            # kv pairs: batch the two row-tiled score pairs together, then the
            # exps, then the pO matmuls -> half as many PE tiling-mode
            # switches.  The pO block of pair p is deferred until after pair
            # p+1's scores are issued (software pipeline), so the PE always
            # has score/weave work in flight while ACT computes pair p's exps
            # -- this matters most at t>=6 where the weave queue is empty.
            def emit_pO(kvs, Es):
                for kv, E in zip(kvs, Es):
                    for hi in range(2):
                        nc.tensor.matmul(
                            pO[hi],
                            v_sb[kv][:, (h0 + hi) * 65 : (h0 + hi) * 65 + 65],
                            E[:, hi, :],
                            start=(kv == 0),
                            stop=(kv == KVT - 1),
                        )

            prev = None
            for kvp in range(KVT // 2):
                kvs = (2 * kvp, 2 * kvp + 1)
                if t == 0 and half == 0:
                    for kv in kvs:
                        emit_C_chain(kv, 0)
                pSs = [emit_scores(t, half, kv, kT_t) for kv in kvs]
                Es = []
                for kv, pS in zip(kvs, pSs):
                    if t >= 6 and kv % 2 == 1:
                        # late head-pairs have no weave chains left; pairs run
                        # at ACT cadence (2 serial exps).  Offload the second
                        # exp to the (idle-there) DVE via the Schraudolph
                        # bf16-bits trick so the two exps run in parallel.
                        Ei = Ep.tile([128, 2, 512], i16, name="E", tag="E")
                        nc.vector.tensor_scalar(
                            Ei.rearrange("p h c -> p (h c)"),
                            pS.rearrange("p h c -> p (h c)"),
                            SCH_A,
                            SCH_B,
                            op0=AluOpType.mult,
                            op1=AluOpType.add,
                        )
                        E = Ei.bitcast(bf16)
                    else:
                        E = Ep.tile([128, 2, 512], bf16, name="E", tag="E")
                        nc.scalar.activation(
                            E.rearrange("p h c -> p (h c)"),
                            pS.rearrange("p h c -> p (h c)"),
                            FT.Exp,
                            scale=SCALE,
                        )
                    Es.append(E)
                if prev is not None:
                    emit_pO(*prev)
                prev = (kvs, Es)
                if wq_queue:
                    wq_queue.pop(0)()
            emit_pO(*prev)

            # normalization: copy psum out; move the ones-column row to
            # partition 0 with a standard copy (custom-DVE ops corrupt on
            # shifted reads, standard ops are fine), approx-reciprocal there,
            # Pool partition-broadcast to 64 rows, scale.  No PE matmuls.
            oc = normp.tile([65, 2, 512], f32r, name="oc", tag="oc")
            for hi in range(2):
                nc.vector.tensor_copy(oc[:, hi, :], pO[hi])
            dcp = normp.tile([1, 2, 512], f32, name="dcp", tag="dcp")
            nc.vector.tensor_copy(dcp, oc[64:65, :, :].bitcast(f32))
            rcp = normp.tile([1, 2, 512], f32, name="rcp", tag="rcp")
            nc.vector.reciprocal_approx_fast(rcp, dcp)
            rb = normp.tile([64, 2, 512], f32, name="rb", tag="rb")
            nc.gpsimd.partition_broadcast(rb[:], rcp[:])
            for hi in range(2):
                dst = OT[t][hi * 64 : hi * 64 + 64, lo : lo + 512]
                nc.vector.tensor_tensor(
                    dst, oc[0:64, hi, :], rb[:, hi, :], op=AluOpType.mult
                )
        while wq_queue:
            wq_queue.pop(0)()

    # SBUF LIFO: normp, Ep, kTp, qryTp, wqkp, wfp, vp, ctxTp, qTp
    normp.release()
    Ep.release()
    kTp.release()
    qryTp.release()
    wqkp.release()
    wfp.release()
    vp.release()
    ctxTp.release()
    qTp.release()
    # PSUM LIFO: psO, psS, psK, wvx
    psO.release()
    psS.release()
    psK.release()
    wvx.release()

    # ---- Phase E: out = O @ w_out + b_out ----
    with (
        tc.tile_pool(name="osb", bufs=3) as osbp,
        tc.tile_pool(name="psE", bufs=4, space="PSUM") as psE,
    ):
        for m in range(NQT):
            o_sb = osbp.tile([128, QD], f32, name="osb", tag="osb")
            for half in range(2):
                lo = half * 512
                ps = psE.tile([128, 512], f32, name="ps_e", tag="ps_e")
                for i in range(IT):
                    nc.tensor.matmul(
                        ps,
                        OT[i][:, m * 128 : (m + 1) * 128],
                        wo[i][:, lo : lo + 512],
                        start=(i == 0),
                        stop=(i == IT - 1),
                    )
                nc.vector.tensor_tensor(
                    o_sb[:, lo : lo + 512],
                    ps,
                    bias_bc[:, lo : lo + 512],
                    op=AluOpType.add,
                )
            nc.gpsimd.dma_start(out[m * 128 : (m + 1) * 128, :], o_sb)

    wvop.release()
    OTp.release()
    const.release()


def build(reps=1):
    nc = bacc.Bacc("TRN2", target_bir_lowering=False, debug=False)
    T = declare(nc)
    with tile.TileContext(nc) as tc:
        for _ in range(reps):
            emit(nc, tc, T)
    nc.compile()
    return nc


_nc_cache = None


def _get_nc():
    global _nc_cache
    if _nc_cache is None:
        _nc_cache = build()
    return _nc_cache


def kernel(query, context, w_q, w_kv, w_out, b_out, **run_kwargs):
    nc = _get_nc()
    query = np.ascontiguousarray(query, dtype=np.float32)
    context = np.ascontiguousarray(context, dtype=np.float32)
    shared = {
        "w_q": np.ascontiguousarray(w_q, dtype=np.float32),
        "w_kv": np.ascontiguousarray(w_kv, dtype=np.float32),
        "w_out": np.ascontiguousarray(w_out, dtype=np.float32),
        "b_out": np.ascontiguousarray(b_out, dtype=np.float32),
    }
    in_maps = [
        {"query": query[b], "context": context[b], **shared} for b in range(B)
    ]
    res = run_bass_kernel_spmd(nc, in_maps, core_ids=list(range(B)), **run_kwargs)
    out = np.stack([res.results[b]["out"] for b in range(B)])
    if run_kwargs:
        kernel.last_result = res
    return out

